# revision 8
# baseline (speedup 1.0000x reference)
"""Causal attention (B=1, T=4096, C=768, H=12, D=64) on 8 trn2 NeuronCores.

v2: ACT-bound redesign. Sharding as v1 (32 blocks of 128 rows; core r owns
blocks {r, r+8, r+16, r+24} as query and K/V rows; XOR-permuted K^T / V'
shard exchange via remote_dma_broadcast, masks absorb the permutation).

Differences vs v1:
- One wide exp per (head, slot): the 4 causal-class score matmuls land in a
  contiguous 3-bank PSUM region ([0:512],[512:896],[896:1024],[1024:1280])
  so softmax is a single [128,1280] Activation instead of 3 narrow ones.
- Every non-exp op leaves the scalar engine: x^T copies and Q/K bias adds
  run on DVE (tensor_scalar_add), V/proj biases stay on DVE.
- Remote-DMA descriptor generation is issued at kernel start (Pool is idle
  then); trigger_dma fires the K wave as soon as K^T is computed and the V
  wave after V', so transfers overlap phase-1 compute.
- Split gates: scores matmuls wait rsem>=2 (K wave), ctx matmuls rsem>=4
  (V wave). Per head: 8 scores/exp windows then 8 ctx batches, with a deep
  et pool so ACT can run ahead of the V exchange.
"""

import numpy as np

T = 4096
C = 768
H = 12
D = 64
J3 = 3 * C          # 2304
NCORES = 8
QB = 128            # query block rows
NQB = T // QB       # 32
SLOTS = 4           # owned query blocks per core (classes m=0..3)
OWN = SLOTS * QB    # 512
CCH = C // 128      # 6 contraction chunks

# class m -> offset of its score strip inside the [128, 1280] region
OFFS = {0: 0, 1: 512, 3: 896, 2: 1024}

_PROGRAM = None


def _blocks(c):
    return [c, 8 + c, 16 + c, 24 + c]


def _build_masks(r):
    """[128, 8*128] f32: mask[k, i*128+q] for diagonal-class key slot i.

    Receiver r's slot i holds sender j = r^i. Diagonal class m==s: key block
    8m+j vs query block 8m+r -> keep iff (r-j)*128 + q - k >= 0.
    """
    k = np.arange(128)[:, None]
    q = np.arange(128)[None, :]
    cols = []
    for i in range(8):
        j = r ^ i
        blk = (((r - j) * 128 + q - k) >= 0).astype(np.float32)
        cols.append(np.concatenate([blk, blk], axis=1))  # doubled: m3+m2 merge
    return np.concatenate(cols, axis=1)


def _build_program(repeat=1):
    import concourse.bass as bass
    import concourse.tile as tile
    from concourse import bacc, mybir
    from concourse.bass import _add_dep_helper
    from concourse.masks import make_identity
    from contextlib import ExitStack

    f32 = mybir.dt.float32
    bf16 = mybir.dt.bfloat16

    nc = bacc.Bacc(
        "TRN2", target_bir_lowering=False, debug=False,
        enable_asserts=False, num_devices=NCORES,
    )

    xo_d = nc.dram_tensor("x_own", [OWN, C], f32, kind="ExternalInput")
    wa_d = nc.dram_tensor("W_attn", [C, J3], bf16, kind="ExternalInput")
    ba_d = nc.dram_tensor("b_attn", [J3], f32, kind="ExternalInput")
    wp_d = nc.dram_tensor("W_proj", [C, C], bf16, kind="ExternalInput")
    bp_d = nc.dram_tensor("b_proj", [C], f32, kind="ExternalInput")
    mk_d = nc.dram_tensor("masks", [128, 8 * 256], bf16, kind="ExternalInput")
    y_d = nc.dram_tensor("y_own", [OWN, C], f32, kind="ExternalOutput")

    fake_credits = []

    with tile.TileContext(nc) as tc:
        for it in range(repeat):
            _emit_once(nc, tc, it, xo_d, wa_d, ba_d, wp_d, bp_d, mk_d, y_d,
                       fake_credits)

    nc.compile()

    # Strip the dry-run-only rsem credits: on hardware the recv gates must be
    # satisfied by the peers' remote increments alone.
    for fake_credit, rsem in fake_credits:
        si = fake_credit.ins.sync_info
        assert si is not None and si.on_update
        kept = [u for u in si.on_update if u.id != rsem.num]
        assert len(kept) < len(si.on_update), "rsem credit not found to strip"
        si.on_update = kept
    return nc


def _emit_once(nc, tc, it, xo_d, wa_d, ba_d, wp_d, bp_d, mk_d, y_d,
               fake_credits):
    import concourse.tile as tile
    from concourse import mybir
    from concourse.bass import _add_dep_helper
    from concourse.masks import make_identity
    from contextlib import ExitStack

    f32 = mybir.dt.float32
    bf16 = mybir.dt.bfloat16
    AF = mybir.ActivationFunctionType
    ALU = mybir.AluOpType
    sfx = f"_{it}"

    rsems = {i: nc.alloc_semaphore(f"rsem{i}" + sfx)
             for i in range(1, NCORES)}
    lsem = nc.alloc_semaphore("lsem" + sfx)  # local send-completion

    with ExitStack() as ctx:
        per = ctx.enter_context(tc.tile_pool(name="per" + sfx, bufs=1))

        # ---- persistent tiles ----
        wp_sb = per.tile([128, CCH, C], bf16)
        # K^T shards: [d(2 heads), slot i, jc(head pair), class m, 128]
        kt_recv = per.tile([128, NCORES, CCH, SLOTS, 128], bf16)
        # V' shards: [k, slot i, class m, head, 65]; [...,64] = 1.0
        v_recv = per.tile([128, NCORES, SLOTS, H, D + 1], bf16)
        qt_sb = per.tile([128, CCH, OWN], bf16)
        ctx_sb = per.tile([128, CCH, OWN], bf16)
        mask_sb = per.tile([128, 8 * 256], bf16)
        ba_sb = per.tile([128, J3 // 128], f32)
        vb_bc = per.tile([128, C], f32)
        pb_bc = per.tile([128, C], f32)
        ident = per.tile([128, 128], f32)
        vb_st = per.tile([1, C], f32)
        pb_st = per.tile([1, C], f32)

        make_identity(nc, ident[:])
        # ones column of own V' shard (slot 0); peers receive it via bcast
        nc.gpsimd.memset(v_recv[:, 0, :, :, D:D + 1], 1.0)

        # ---- small DMAs + x ----
        nc.sync.dma_start(ba_sb[:], ba_d.rearrange("(a p) -> p a", p=128))
        nc.sync.dma_start(vb_st[:], ba_d[2 * C:3 * C][None, :])
        if it > 0:
            # benchmark-only serialization: body `it` consumes (x0) the
            # previous body's output, so repeats can't overlap or fold.
            # On the ACT hwdge queue so the SP queue stays pure-input and
            # later bodies' weight loads prefetch under this body's compute.
            chain = per.tile([1, C], f32)
            nc.gpsimd.dma_start(chain[:], y_d[0:1, :])
            nc.vector.tensor_scalar_mul(chain[:], chain[:], 0.0)
            nc.vector.tensor_add(vb_st[:], vb_st[:], chain[:])
        nc.sync.dma_start(pb_st[:], bp_d[None, :])
        nc.sync.dma_start(mask_sb[:], mk_d[:])

        # ---- early desc-gen: K wave (slot i gets sender r^i) ----
        kt_own = kt_recv[:, 0, :, :, :].rearrange("p a b c -> p (a b c)")
        v_own = v_recv[:, 0, :, :, :].rearrange("p a b c -> p (a b c)")
        for i in range(1, NCORES):
            rd = [None] * 8
            rd[i] = (0, i)
            nc.gpsimd.remote_dma_broadcast(
                kt_recv[:, i, :, :, :].rearrange("p a b c -> p (a b c)"),
                kt_own, rsems[i], lsem, rdests=rd)

        nc.gpsimd.partition_broadcast(vb_bc[:], vb_st[:])
        nc.gpsimd.partition_broadcast(pb_bc[:], pb_st[:])

        # score-region PSUM pool (2 x 3 banks) + pass-A et pool live for the
        # whole kernel so own-slot attention can interleave into phase 1
        p2ps = ctx.enter_context(
            tc.tile_pool(name="p2ps" + sfx, bufs=2, space="PSUM"))
        etA = ctx.enter_context(tc.tile_pool(name="etA" + sfx, bufs=8))

        kgate = {}
        vgate = {}
        scale = 1.0 / float(np.sqrt(D))

        def dep_on(inst, gate):
            # same-engine (PE) ordering edge: the gate precedes every
            # attention matmul reading the remote shard
            _add_dep_helper(inst.ins, gate.ins, sync=False,
                            reason="attention reads remote K/V shard")

        def pair_scores(ha, hb, i, et_pool):
            """Score windows for both heads of a pair at slot i.

            The heads live on disjoint partition halves (po 0 / 64), so
            their K=64-contraction matmuls target disjoint PE row groups
            (tile_position auto-derived from base_partition) and execute
            concurrently when interleaved in the issue stream.
            """
            jc = ha // 2
            reg_a = p2ps.tile([128, 1536], f32, tag="reg")
            reg_b = p2ps.tile([128, 1536], f32, tag="reg")
            regs = (reg_a, reg_b)
            for m in range(SLOTS):
                n = OWN - 128 * m
                for h, reg in zip((ha, hb), regs):
                    po = 64 * (h % 2)
                    mm = nc.tensor.matmul(
                        reg[:, OFFS[m]:OFFS[m] + n],
                        kt_recv[po:po + D, i, jc, m, :],
                        qt_sb[po:po + D, jc, 128 * m:OWN],
                        start=True, stop=True,
                    )
                    if i != 0:
                        dep_on(mm, kgate[i])
            out = []
            for h, reg in zip((ha, hb), regs):
                et = et_pool.tile([128, 1280], bf16, tag="et")
                nc.scalar.activation(
                    et[:], reg[:, 0:1280], AF.Exp, scale=scale)
                for lo, wdt in ((OFFS[0], 128), (OFFS[1], 128),
                                (OFFS[3], 256)):
                    # diagonal strips (query slot == class); m3+m2 strips
                    # are adjacent at [896:1152] -> one op on a doubled mask
                    nc.vector.tensor_mul(
                        et[:, lo:lo + wdt], et[:, lo:lo + wdt],
                        mask_sb[:, i * 256:i * 256 + wdt])
                ets[h][i] = et
                out.append(et)
            return out

        ets = {h: {} for h in range(H)}

        # ---- phase 1: own-row QKV projection ----
        with tc.tile_pool(name="p1" + sfx, bufs=1) as p1:
            # x first on the DMA queue (transposes gate everything), then
            # W_attn in K / Q / V column groups, W_proj last (phase 3 only)
            xst = p1.tile([128, SLOTS, C], f32, tag="xst")
            xre = xo_d.rearrange("(g p) c -> p g c", p=128)
            for cc in range(CCH):
                nc.sync.dma_start(
                    xst[:, :, cc * 128:(cc + 1) * 128],
                    xre[:, :, cc * 128:(cc + 1) * 128])
            wa_sb = p1.tile([128, CCH, J3], bf16, tag="wa_sb")
            for cg in (1, 0, 2):  # column group: k, q, v
                for cc in range(CCH):
                    nc.sync.dma_start(
                        wa_sb[:, cc, cg * C:(cg + 1) * C],
                        wa_d[cc * 128:(cc + 1) * 128, cg * C:(cg + 1) * C])
            for cc in range(CCH):
                nc.sync.dma_start(
                    wp_sb[:, cc, :], wp_d[cc * 128:(cc + 1) * 128, :])
            xt = p1.tile([128, CCH, OWN], bf16, tag="xt")

            with tc.tile_pool(name="p1ps" + sfx, bufs=2,
                              space="PSUM") as p1ps:
                for cc in range(CCH):
                    ps_t = p1ps.tile([128, OWN], f32, tag="ps")
                    for g in range(SLOTS):
                        nc.tensor.transpose(
                            ps_t[:, g * 128:(g + 1) * 128],
                            xst[:, g, cc * 128:(cc + 1) * 128],
                            ident[:],
                        )
                    nc.vector.tensor_scalar_add(xt[:, cc, :], ps_t[:], 0.0)

                # K^T shard -> kt_recv slot 0
                kt_writes = []
                for jc in range(CCH):
                    ps_k = p1ps.tile([128, OWN], f32, tag="ps")
                    for cc in range(CCH):
                        nc.tensor.matmul(
                            ps_k[:],
                            wa_sb[:, cc,
                                  (CCH + jc) * 128:(CCH + jc + 1) * 128],
                            xt[:, cc, :],
                            start=(cc == 0), stop=(cc == CCH - 1),
                        )
                    w = nc.vector.tensor_scalar_add(
                        kt_recv[:, 0, jc, :, :].rearrange("p a b -> p (a b)"),
                        ps_k[:], ba_sb[:, CCH + jc:CCH + jc + 1])
                    kt_writes.append(w)
                # fire the K wave as soon as the shard is in SBUF
                trigK = nc.gpsimd.trigger_dma(count=None)
                for w in kt_writes:
                    _add_dep_helper(trigK.ins, w.ins, sync=True,
                                    reason="K shard data ready")

                # V wave desc-gen now (Pool is past the K preps)
                for i in range(1, NCORES):
                    rd = [None] * 8
                    rd[i] = (0, i)
                    nc.gpsimd.remote_dma_broadcast(
                        v_recv[:, i, :, :, :].rearrange(
                            "p a b c -> p (a b c)"),
                        v_own, rsems[i], lsem, rdests=rd)

                # Q^T
                for jc in range(CCH):
                    ps_q = p1ps.tile([128, OWN], f32, tag="ps")
                    for cc in range(CCH):
                        nc.tensor.matmul(
                            ps_q[:],
                            wa_sb[:, cc, jc * 128:(jc + 1) * 128],
                            xt[:, cc, :],
                            start=(cc == 0), stop=(cc == CCH - 1),
                        )
                    nc.vector.tensor_scalar_add(
                        qt_sb[:, jc, :], ps_q[:], ba_sb[:, jc:jc + 1])

            # pass A: own-slot windows for h0..h7 — ungated ACT runway that
            # starts while V is still being computed and covers the K-wave
            # transfer latency
            for hp in range(4):
                pair_scores(2 * hp, 2 * hp + 1, 0, etA)

            # V shard -> v_recv slot 0
            v_writes = []
            with tc.tile_pool(name="p1pv" + sfx, bufs=2,
                              space="PSUM") as p1pv:
                for g in range(SLOTS):
                    for e2 in range(2):
                        ps_v = p1pv.tile([128, 384], f32, tag="ps_v")
                        for cc in range(CCH):
                            nc.tensor.matmul(
                                ps_v[:],
                                xt[:, cc, g * 128:(g + 1) * 128],
                                wa_sb[:, cc,
                                      2 * C + 384 * e2:2 * C + 384 * (e2 + 1)],
                                start=(cc == 0), stop=(cc == CCH - 1),
                            )
                        w = nc.vector.tensor_tensor(
                            v_recv[:, 0, g, 6 * e2:6 * (e2 + 1), 0:D],
                            ps_v[:].rearrange("p (h d) -> p h d", d=D),
                            vb_bc[:, 384 * e2:384 * (e2 + 1)].rearrange(
                                "p (h d) -> p h d", d=D),
                            op=ALU.add,
                        )
                        v_writes.append(w)
            trigV = nc.gpsimd.trigger_dma(count=None)
            for w in v_writes:
                _add_dep_helper(trigV.ins, w.ins, sync=True,
                                reason="V shard data ready before transfer")

        # ---- per-slot recv gates ----
        # Each wave bumps the receiver's rsem by 2 (16 // n_dests). Scores
        # need only the K wave (>=2); ctx also needs V' (>=4). Dry-run-only
        # credits (+4) keep the Tile scheduling sim from deadlocking; they
        # are stripped from the compiled program.
        def emit_gate(i, thresh):
            w = nc.tensor.wait_ge(rsems[i], thresh)
            _add_dep_helper(w.ins, credits[i].ins, sync=True,
                            reason="recv gate after sends initiated")
            return w

        credits = {}
        for i in range(1, NCORES):
            fc = nc.gpsimd.nop(nofuse=True).then_inc(rsems[i], 4)
            _add_dep_helper(fc.ins, trigV.ins, sync=False,
                            reason="credit after trigger")
            fake_credits.append((fc, rsems[i]))
            credits[i] = fc

        # ---- phase 2: attention ----
        with (
            tc.tile_pool(name="p2" + sfx, bufs=14) as p2,
            tc.tile_pool(name="p2n" + sfx, bufs=2) as p2n,
            tc.tile_pool(name="p2pc" + sfx, bufs=2, space="PSUM") as p2pc,
        ):
            def ctx_batch(h, i, ps_ctx):
                et = ets[h].pop(i)
                for m in range(SLOTS):
                    n = OWN - 128 * m
                    mm = nc.tensor.matmul(
                        ps_ctx[:, 128 * m:OWN],
                        v_recv[:, i, m, h, :],
                        et[:, OFFS[m]:OFFS[m] + n],
                        start=(i == 0 and m == 0),
                        stop=(i == NCORES - 1 and m == SLOTS - 1),
                        skip_group_check=True,
                    )
                    if i != 0:
                        dep_on(mm, vgate[i])

            def normalize(h, ps_ctx):
                jc = h // 2
                po = 64 * (h % 2)
                rec = p2n.tile([1, OWN], f32, tag="rec")
                nc.vector.reciprocal(rec[:], ps_ctx[D:D + 1, :])
                rb = p2n.tile([D, OWN], f32, tag="rb")
                nc.gpsimd.partition_broadcast(rb[:], rec[:])
                nc.vector.tensor_mul(
                    ctx_sb[po:po + D, jc, :], ps_ctx[0:D, :], rb[:])

            # pass B: per head pair — remaining scores windows, then the
            # ctx batches. Head pairs 4/5 get their own-slot windows here
            # (still from etA: its bufs recycle once early ctx consumes).
            last = H // 2 - 1
            for hp in range(H // 2):
                ha, hb = 2 * hp, 2 * hp + 1
                if hp >= 4:
                    pair_scores(ha, hb, 0, etA)
                ps_ctx_a = p2pc.tile([D + 1, OWN], f32, tag="ps_ctx")
                ps_ctx_b = p2pc.tile([D + 1, OWN], f32, tag="ps_ctx")
                if hp < last:
                    for i in range(1, NCORES):
                        if hp == 0:
                            kgate[i] = emit_gate(i, 2)
                        pair_scores(ha, hb, i, p2)
                    # ctx batches (V ones-column -> row 64 = softmax denom)
                    for i in range(NCORES):
                        if hp == 0 and i >= 1:
                            vgate[i] = emit_gate(i, 4)
                        ctx_batch(ha, i, ps_ctx_a)
                        ctx_batch(hb, i, ps_ctx_b)
                else:
                    # final pair: interleave ctx under the scores/exp
                    # pipeline (V long since arrived) so the serial tail
                    # after the last exp is just one ctx batch + proj
                    for i in range(1, NCORES):
                        pair_scores(ha, hb, i, p2)
                        ctx_batch(ha, i - 1, ps_ctx_a)
                        ctx_batch(hb, i - 1, ps_ctx_b)
                    ctx_batch(ha, NCORES - 1, ps_ctx_a)
                    ctx_batch(hb, NCORES - 1, ps_ctx_b)
                normalize(ha, ps_ctx_a)
                normalize(hb, ps_ctx_b)

        # ---- phase 3: output projection ----
        with (
            tc.tile_pool(name="p3" + sfx, bufs=2) as p3,
            tc.tile_pool(name="p3ps" + sfx, bufs=2, space="PSUM") as p3ps,
        ):
            for ts in range(OWN // 128):
                y_sb = p3.tile([128, C], f32, tag="y_sb")
                for e2 in range(2):
                    ps_o = p3ps.tile([128, 384], f32, tag="ps_o")
                    for cc in range(CCH):
                        nc.tensor.matmul(
                            ps_o[:],
                            ctx_sb[:, cc, ts * 128:(ts + 1) * 128],
                            wp_sb[:, cc, 384 * e2:384 * (e2 + 1)],
                            start=(cc == 0), stop=(cc == CCH - 1),
                        )
                    nc.vector.tensor_tensor(
                        y_sb[:, 384 * e2:384 * (e2 + 1)], ps_o[:],
                        pb_bc[:, 384 * e2:384 * (e2 + 1)], op=ALU.add,
                    )
                nc.gpsimd.dma_start(
                    y_d[ts * 128:(ts + 1) * 128, :], y_sb[:])


_RUNNER = None


def _get_runner():
    """Build the 8-core PJRT executable once; returns f(in_maps) -> results."""
    global _PROGRAM, _RUNNER
    if _RUNNER is not None:
        return _RUNNER
    import jax
    from jax.sharding import Mesh, PartitionSpec
    from jax.experimental.shard_map import shard_map
    from concourse import mybir
    from concourse.bass2jax import (
        _bass_exec_p, install_neuronx_cc_hook, partition_id_tensor)

    if _PROGRAM is None:
        _PROGRAM = _build_program()
    nc = _PROGRAM
    install_neuronx_cc_hook()

    partition_name = (
        nc.partition_id_tensor.name if nc.partition_id_tensor else None)
    in_names, out_names, out_avals, zero_outs = [], [], [], []
    for alloc in nc.m.functions[0].allocations:
        if not isinstance(alloc, mybir.MemoryLocationSet):
            continue
        name = alloc.memorylocations[0].name
        if alloc.kind == "ExternalInput":
            if name == partition_name:
                continue
            in_names.append(name)
        elif alloc.kind == "ExternalOutput":
            shape = tuple(alloc.tensor_shape)
            dtype = mybir.dt.np(alloc.dtype)
            out_names.append(name)
            out_avals.append(jax.core.ShapedArray(shape, dtype))
            zero_outs.append(np.zeros(shape, dtype))
    n_params = len(in_names)
    all_names = in_names + out_names
    if partition_name is not None:
        all_names = all_names + [partition_name]
    donate = tuple(range(n_params, n_params + len(out_names)))

    def _body(*args):
        operands = list(args)
        if partition_name is not None:
            operands.append(partition_id_tensor())
        outs = _bass_exec_p.bind(
            *operands,
            out_avals=tuple(out_avals),
            in_names=tuple(all_names),
            out_names=tuple(out_names),
            lowering_input_output_aliases=(),
            sim_require_finite=True,
            sim_require_nnan=True,
            nc=nc,
        )
        return tuple(outs)

    devices = jax.devices()[:NCORES]
    mesh = Mesh(np.asarray(devices), ("core",))
    specs = (PartitionSpec("core"),) * (n_params + len(out_names))
    sharded = jax.jit(
        shard_map(_body, mesh=mesh, in_specs=specs,
                  out_specs=(PartitionSpec("core"),) * len(out_names),
                  check_rep=False),
        donate_argnums=donate, keep_unused=True,
    )

    def run(in_maps):
        concat_in = [
            np.concatenate([np.asarray(m[name]) for m in in_maps], axis=0)
            for name in in_names
        ]
        concat_zeros = [
            np.zeros((NCORES * z.shape[0], *z.shape[1:]), z.dtype)
            for z in zero_outs
        ]
        out_arrs = jax.block_until_ready(sharded(*concat_in, *concat_zeros))
        return [
            {name: np.asarray(out_arrs[i]).reshape(NCORES, *out_avals[i].shape)[c]
             for i, name in enumerate(out_names)}
            for c in range(NCORES)
        ]

    _RUNNER = run
    run._parts = (_body, in_names, out_names, out_avals, zero_outs, mesh)
    return run


def _make_timed_fn(nc, in_maps):
    """Compile one-dispatch callable with device-resident inputs."""
    import jax
    from jax.sharding import Mesh, NamedSharding, PartitionSpec
    from jax.experimental.shard_map import shard_map
    from concourse import mybir
    from concourse.bass2jax import (
        _bass_exec_p, install_neuronx_cc_hook, partition_id_tensor)

    install_neuronx_cc_hook()
    partition_name = (
        nc.partition_id_tensor.name if nc.partition_id_tensor else None)
    in_names, out_names, out_avals, zero_outs = [], [], [], []
    for alloc in nc.m.functions[0].allocations:
        if not isinstance(alloc, mybir.MemoryLocationSet):
            continue
        name = alloc.memorylocations[0].name
        if alloc.kind == "ExternalInput":
            if name == partition_name:
                continue
            in_names.append(name)
        elif alloc.kind == "ExternalOutput":
            out_names.append(name)
            out_avals.append(jax.core.ShapedArray(
                tuple(alloc.tensor_shape), mybir.dt.np(alloc.dtype)))
            zero_outs.append(np.zeros(
                tuple(alloc.tensor_shape), mybir.dt.np(alloc.dtype)))
    all_names = in_names + out_names
    if partition_name is not None:
        all_names = all_names + [partition_name]

    def _body(*args):
        operands = list(args)
        if partition_name is not None:
            operands.append(partition_id_tensor())
        return tuple(_bass_exec_p.bind(
            *operands,
            out_avals=tuple(out_avals),
            in_names=tuple(all_names),
            out_names=tuple(out_names),
            lowering_input_output_aliases=(),
            sim_require_finite=True, sim_require_nnan=True, nc=nc,
        ))

    devices = jax.devices()[:NCORES]
    mesh = Mesh(np.asarray(devices), ("core",))
    fn = jax.jit(shard_map(
        _body, mesh=mesh,
        in_specs=(PartitionSpec("core"),) * (len(in_names) + len(zero_outs)),
        out_specs=(PartitionSpec("core"),) * len(out_names),
        check_rep=False))
    sh = NamedSharding(mesh, PartitionSpec("core"))
    concat_in = [
        jax.device_put(np.concatenate(
            [np.asarray(m[name]) for m in in_maps], axis=0), sh)
        for name in in_names
    ]
    concat_zeros = [
        jax.device_put(
            np.zeros((NCORES * z.shape[0], *z.shape[1:]), z.dtype), sh)
        for z in zero_outs
    ]
    jax.block_until_ready(fn(*concat_in, *concat_zeros))  # warm/compile

    def call():
        import time
        t0 = time.perf_counter()
        jax.block_until_ready(fn(*concat_in, *concat_zeros))
        return time.perf_counter() - t0
    return call


def _bench_device_time(in_maps, iters=20, n_rep=8):
    """Per-execution device time: the kernel body emitted n_rep times in one
    program vs once. The n_rep bodies are serialized on-device (DRAM chain),
    so min(walls_n) - min(walls_1) is a tight estimate of (n_rep-1) execs
    that is robust to dispatch-RTT congestion spikes."""
    global _PROGRAM
    if _PROGRAM is None:
        _PROGRAM = _build_program()
    call1 = _make_timed_fn(_PROGRAM, in_maps)
    calln = _make_timed_fn(_build_program(n_rep), in_maps)

    w1, wn = [], []
    for _ in range(max(4, iters // 3)):
        call1()  # absorb NEFF swap from previous block
        w1.extend(call1() for _ in range(6))
        calln()
        wn.extend(calln() for _ in range(6))
    per_exec = (min(wn) - min(w1)) / (n_rep - 1)
    deltas = sorted(wn)[:3]
    return per_exec, {
        "rep1_min": min(w1),
        f"rep{n_rep}_min": min(wn),
        "delta_med": per_exec,
        "rep1_p25": sorted(w1)[len(w1) // 4],
        f"rep{n_rep}_p25": sorted(wn)[len(wn) // 4],
    }


def _make_in_maps(x2, wa, ba, wp, bp):
    import ml_dtypes
    bf = ml_dtypes.bfloat16
    wa16 = np.ascontiguousarray(wa.astype(bf))
    wp16 = np.ascontiguousarray(wp.astype(bf))
    in_maps = []
    for c in range(NCORES):
        xo = np.concatenate([x2[128 * b:128 * (b + 1)] for b in _blocks(c)], 0)
        in_maps.append({
            "x_own": np.ascontiguousarray(xo),
            "W_attn": wa16, "b_attn": ba, "W_proj": wp16, "b_proj": bp,
            "masks": _build_masks(c).astype(bf),
        })
    return in_maps


def kernel(x, W_attn, b_attn, W_proj, b_proj):
    x2 = np.ascontiguousarray(np.asarray(x, dtype=np.float32).reshape(T, C))
    wa = np.ascontiguousarray(np.asarray(W_attn, dtype=np.float32))
    ba = np.ascontiguousarray(np.asarray(b_attn, dtype=np.float32))
    wp = np.ascontiguousarray(np.asarray(W_proj, dtype=np.float32))
    bp = np.ascontiguousarray(np.asarray(b_proj, dtype=np.float32))

    run = _get_runner()
    res = run(_make_in_maps(x2, wa, ba, wp, bp))

    y = np.empty((T, C), dtype=np.float32)
    for c in range(NCORES):
        yo = res[c]["y_own"]
        for s, b in enumerate(_blocks(c)):
            y[128 * b:128 * (b + 1)] = yo[128 * s:128 * (s + 1)]
    return y.reshape(1, T, C)


# revision 18
# speedup vs baseline: 1.3083x; 1.3083x over previous
"""Causal attention (B=1, T=4096, C=768, H=12, D=64) on 8 trn2 NeuronCores.

v2: ACT-bound redesign. Sharding as v1 (32 blocks of 128 rows; core r owns
blocks {r, r+8, r+16, r+24} as query and K/V rows; XOR-permuted K^T / V'
shard exchange via remote_dma_broadcast, masks absorb the permutation).

Differences vs v1:
- One wide exp per (head, slot): the 4 causal-class score matmuls land in a
  contiguous 3-bank PSUM region ([0:512],[512:896],[896:1024],[1024:1280])
  so softmax is a single [128,1280] Activation instead of 3 narrow ones.
- Every non-exp op leaves the scalar engine: x^T copies and Q/K bias adds
  run on DVE (tensor_scalar_add), V/proj biases stay on DVE.
- Remote-DMA descriptor generation is issued at kernel start (Pool is idle
  then); trigger_dma fires the K wave as soon as K^T is computed and the V
  wave after V', so transfers overlap phase-1 compute.
- Split gates: scores matmuls wait rsem>=2 (K wave), ctx matmuls rsem>=4
  (V wave). Per head: 8 scores/exp windows then 8 ctx batches, with a deep
  et pool so ACT can run ahead of the V exchange.
"""

import numpy as np

T = 4096
C = 768
H = 12
D = 64
J3 = 3 * C          # 2304
NCORES = 8
QB = 128            # query block rows
NQB = T // QB       # 32
SLOTS = 4           # owned query blocks per core (classes m=0..3)
OWN = SLOTS * QB    # 512
CCH = C // 128      # 6 contraction chunks

# class m -> offset of its score strip inside the [128, 1280] region
OFFS = {0: 0, 1: 512, 3: 896, 2: 1024}

_PROGRAM = None


def _blocks(c):
    return [c, 8 + c, 16 + c, 24 + c]


def _build_masks(r):
    """[128, 8*128] f32: mask[k, i*128+q] for diagonal-class key slot i.

    Receiver r's slot i holds sender j = r^i. Diagonal class m==s: key block
    8m+j vs query block 8m+r -> keep iff (r-j)*128 + q - k >= 0.
    """
    k = np.arange(128)[:, None]
    q = np.arange(128)[None, :]
    cols = []
    for i in range(8):
        j = r ^ i
        blk = (((r - j) * 128 + q - k) >= 0).astype(np.float32)
        cols.append(np.concatenate([blk, blk], axis=1))  # doubled: m3+m2 merge
    return np.concatenate(cols, axis=1)


def _build_program(repeat=1):
    import concourse.bass as bass
    import concourse.tile as tile
    from concourse import bacc, mybir
    from concourse.bass import _add_dep_helper
    from concourse.masks import make_identity
    from contextlib import ExitStack

    f32 = mybir.dt.float32
    bf16 = mybir.dt.bfloat16

    nc = bacc.Bacc(
        "TRN2", target_bir_lowering=False, debug=False,
        enable_asserts=False, num_devices=NCORES,
    )

    xo_d = nc.dram_tensor("x_own", [OWN, C], f32, kind="ExternalInput")
    wa_d = nc.dram_tensor("W_attn", [C, J3], bf16, kind="ExternalInput")
    ba_d = nc.dram_tensor("b_attn", [J3], f32, kind="ExternalInput")
    wp_d = nc.dram_tensor("W_proj", [C, C], bf16, kind="ExternalInput")
    bp_d = nc.dram_tensor("b_proj", [C], f32, kind="ExternalInput")
    mk_d = nc.dram_tensor("masks", [128, 8 * 256], bf16, kind="ExternalInput")
    y_d = nc.dram_tensor("y_own", [OWN, C], f32, kind="ExternalOutput")

    fake_credits = []

    with tile.TileContext(nc) as tc:
        for it in range(repeat):
            _emit_once(nc, tc, it, xo_d, wa_d, ba_d, wp_d, bp_d, mk_d, y_d,
                       fake_credits)

    nc.compile()

    # Strip the dry-run-only rsem credits: on hardware the recv gates must be
    # satisfied by the peers' remote increments alone.
    for fake_credit, rsem in fake_credits:
        si = fake_credit.ins.sync_info
        assert si is not None and si.on_update
        kept = [u for u in si.on_update if u.id != rsem.num]
        assert len(kept) < len(si.on_update), "rsem credit not found to strip"
        si.on_update = kept
    return nc


def _emit_once(nc, tc, it, xo_d, wa_d, ba_d, wp_d, bp_d, mk_d, y_d,
               fake_credits):
    import concourse.tile as tile
    from concourse import mybir
    from concourse.bass import _add_dep_helper
    from concourse.masks import make_identity
    from contextlib import ExitStack

    f32 = mybir.dt.float32
    bf16 = mybir.dt.bfloat16
    AF = mybir.ActivationFunctionType
    ALU = mybir.AluOpType
    sfx = f"_{it}"

    rsems = {i: nc.alloc_semaphore(f"rsem{i}" + sfx)
             for i in range(1, NCORES)}
    lsem = nc.alloc_semaphore("lsem" + sfx)  # local send-completion

    with ExitStack() as ctx:
        per = ctx.enter_context(tc.tile_pool(name="per" + sfx, bufs=1))

        # ---- persistent tiles ----
        wp_sb = per.tile([128, CCH, C], bf16)
        # K^T shards: [d(2 heads), slot i, jc(head pair), class m, 128]
        kt_recv = per.tile([128, NCORES, CCH, SLOTS, 128], bf16)
        # V' shards: [k, slot i, class m, head, 65]; [...,64] = 1.0
        v_recv = per.tile([128, NCORES, SLOTS, H, D + 1], bf16)
        qt_sb = per.tile([128, CCH, OWN], bf16)
        ctx_sb = per.tile([128, CCH, OWN], bf16)
        mask_sb = per.tile([128, 8 * 256], bf16)
        ba_sb = per.tile([128, J3 // 128], f32)
        vb_bc = per.tile([128, C], f32)
        pb_bc = per.tile([128, C], f32)
        ident = per.tile([128, 128], f32)
        vb_st = per.tile([1, C], f32)
        pb_st = per.tile([1, C], f32)

        make_identity(nc, ident[:])
        # ones column of own V' shard (slot 0); peers receive it via bcast
        nc.gpsimd.memset(v_recv[:, 0, :, :, D:D + 1], 1.0)

        # ---- small DMAs + x ----
        nc.sync.dma_start(ba_sb[:], ba_d.rearrange("(a p) -> p a", p=128))
        nc.sync.dma_start(vb_st[:], ba_d[2 * C:3 * C][None, :])
        if it > 0:
            # benchmark-only serialization: body `it` consumes (x0) the
            # previous body's output, so repeats can't overlap or fold.
            # gpsimd queue keeps the SP queue pure-input.
            chain = per.tile([1, C], f32)
            nc.gpsimd.dma_start(chain[:], y_d[0:1, :])
            nc.vector.tensor_scalar_mul(chain[:], chain[:], 0.0)
            nc.vector.tensor_add(vb_st[:], vb_st[:], chain[:])
        nc.sync.dma_start(pb_st[:], bp_d[None, :])
        nc.sync.dma_start(mask_sb[:], mk_d[:])

        # ---- early desc-gen: K wave (slot i gets sender r^i) ----
        kt_own = kt_recv[:, 0, :, :, :].rearrange("p a b c -> p (a b c)")
        v_own = v_recv[:, 0, :, :, :].rearrange("p a b c -> p (a b c)")

        def prep_wave(recv, own, slots):
            for i in slots:
                rd = [None] * 8
                rd[i] = (0, i)
                nc.gpsimd.remote_dma_broadcast(
                    recv[:, i, :, :, :].rearrange("p a b c -> p (a b c)"),
                    own, rsems[i], lsem, rdests=rd)

        prep_wave(kt_recv, kt_own, range(1, NCORES))

        nc.gpsimd.partition_broadcast(vb_bc[:], vb_st[:])
        nc.gpsimd.partition_broadcast(pb_bc[:], pb_st[:])

        # score-region PSUM pool (2 x 3 banks) + pass-A et pool live for the
        # whole kernel so own-slot attention can interleave into phase 1
        p2ps = ctx.enter_context(
            tc.tile_pool(name="p2ps" + sfx, bufs=2, space="PSUM"))
        etA = ctx.enter_context(tc.tile_pool(name="etA" + sfx, bufs=8))

        kgate = {}
        vgate = {}
        scale = 1.0 / float(np.sqrt(D))

        def dep_on(inst, gate):
            # same-engine (PE) ordering edge: the gate precedes every
            # attention matmul reading the remote shard
            _add_dep_helper(inst.ins, gate.ins, sync=False,
                            reason="attention reads remote K/V shard")

        def pair_scores(ha, hb, i, et_pool):
            """Score windows for both heads of a pair at slot i.

            The heads live on disjoint partition halves (po 0 / 64), so
            their K=64-contraction matmuls target disjoint PE row groups
            (tile_position auto-derived from base_partition) and execute
            concurrently when interleaved in the issue stream.
            """
            jc = ha // 2
            reg_a = p2ps.tile([128, 1536], f32, tag="reg")
            reg_b = p2ps.tile([128, 1536], f32, tag="reg")
            regs = (reg_a, reg_b)
            for m in range(SLOTS):
                n = OWN - 128 * m
                for h, reg in zip((ha, hb), regs):
                    po = 64 * (h % 2)
                    if i == 0 and jc < 4:
                        # own-slot K^T comes from the staging tile: it is
                        # written with no WAR against the K-wave desc-gens,
                        # so these windows start ~15us earlier
                        kt_src = stash["kt_st"][po:po + D, jc, m, :]
                    else:
                        kt_src = kt_recv[po:po + D, i, jc, m, :]
                    mm = nc.tensor.matmul(
                        reg[:, OFFS[m]:OFFS[m] + n],
                        kt_src,
                        qt_sb[po:po + D, jc, 128 * m:OWN],
                        start=True, stop=True,
                    )
                    if i != 0:
                        dep_on(mm, kgate[i])
            out = []
            for h, reg in zip((ha, hb), regs):
                et = et_pool.tile([128, 1280], bf16, tag="et")
                nc.scalar.activation(
                    et[:], reg[:, 0:1280], AF.Exp, scale=scale)
                for lo, wdt in ((OFFS[0], 128), (OFFS[1], 128),
                                (OFFS[3], 256)):
                    # diagonal strips (query slot == class); m3+m2 strips
                    # are adjacent at [896:1152] -> one op on a doubled mask
                    nc.vector.tensor_mul(
                        et[:, lo:lo + wdt], et[:, lo:lo + wdt],
                        mask_sb[:, i * 256:i * 256 + wdt])
                ets[h][i] = et
                out.append(et)
            return out

        ets = {h: {} for h in range(H)}
        stash = {}

        # ---- phase 1: own-row QKV projection ----
        with tc.tile_pool(name="p1" + sfx, bufs=1) as p1:
            # x first on the DMA queue (transposes gate everything), then
            # W_attn in K / Q / V column groups, W_proj last (phase 3 only)
            xst = p1.tile([128, SLOTS, C], f32, tag="xst")
            xre = xo_d.rearrange("(g p) c -> p g c", p=128)
            for cc in range(CCH):
                nc.sync.dma_start(
                    xst[:, :, cc * 128:(cc + 1) * 128],
                    xre[:, :, cc * 128:(cc + 1) * 128])
            wa_sb = p1.tile([128, CCH, J3], bf16, tag="wa_sb")
            for cg in (1, 0, 2):  # column group: k, q, v
                for cc in range(CCH):
                    nc.sync.dma_start(
                        wa_sb[:, cc, cg * C:(cg + 1) * C],
                        wa_d[cc * 128:(cc + 1) * 128, cg * C:(cg + 1) * C])
            for cc in range(CCH):
                nc.sync.dma_start(
                    wp_sb[:, cc, :], wp_d[cc * 128:(cc + 1) * 128, :])
            xt = p1.tile([128, CCH, OWN], bf16, tag="xt")

            # staging for K^T chunks jc<4: written WAR-free (the K-wave
            # desc-gens only read kt_recv), feeds pass-A windows immediately
            kt_st = p1.tile([128, 4, SLOTS, 128], bf16, tag="kt_st")
            stash["kt_st"] = kt_st

            with tc.tile_pool(name="p1ps" + sfx, bufs=2,
                              space="PSUM") as p1ps:
                for cc in range(CCH):
                    ps_t = p1ps.tile([128, OWN], f32, tag="ps")
                    for g in range(SLOTS):
                        nc.tensor.transpose(
                            ps_t[:, g * 128:(g + 1) * 128],
                            xst[:, g, cc * 128:(cc + 1) * 128],
                            ident[:],
                        )
                    nc.vector.tensor_scalar_add(xt[:, cc, :], ps_t[:], 0.0)

                # K^T, Q^T and pass-A own-slot windows, chunk-interleaved:
                # each jc enables head pair jc's window right away, so the
                # exp pipeline starts ~chunk-0 time instead of after all of
                # phase 1
                kt_writes = []
                for jc in range(CCH):
                    ps_k = p1ps.tile([128, OWN], f32, tag="ps")
                    for cc in range(CCH):
                        nc.tensor.matmul(
                            ps_k[:],
                            wa_sb[:, cc,
                                  (CCH + jc) * 128:(CCH + jc + 1) * 128],
                            xt[:, cc, :],
                            start=(cc == 0), stop=(cc == CCH - 1),
                        )
                    if jc < 4:
                        nc.vector.tensor_scalar_add(
                            kt_st[:, jc, :, :].rearrange("p a b -> p (a b)"),
                            ps_k[:], ba_sb[:, CCH + jc:CCH + jc + 1])
                    else:
                        w = nc.vector.tensor_scalar_add(
                            kt_recv[:, 0, jc, :, :].rearrange(
                                "p a b -> p (a b)"),
                            ps_k[:], ba_sb[:, CCH + jc:CCH + jc + 1])
                        kt_writes.append(w)
                    ps_q = p1ps.tile([128, OWN], f32, tag="ps")
                    for cc in range(CCH):
                        nc.tensor.matmul(
                            ps_q[:],
                            wa_sb[:, cc, jc * 128:(jc + 1) * 128],
                            xt[:, cc, :],
                            start=(cc == 0), stop=(cc == CCH - 1),
                        )
                    nc.vector.tensor_scalar_add(
                        qt_sb[:, jc, :], ps_q[:], ba_sb[:, jc:jc + 1])
                    if jc < 4:
                        pair_scores(2 * jc, 2 * jc + 1, 0, etA)

                # exchange copies of the staged chunks (these pay the WAR
                # against the K-wave desc-gens, off the critical path)
                for jc in range(4):
                    w = nc.vector.tensor_scalar_add(
                        kt_recv[:, 0, jc, :, :].rearrange("p a b -> p (a b)"),
                        kt_st[:, jc, :, :].rearrange("p a b -> p (a b)"),
                        0.0)
                    kt_writes.append(w)
                # fire the K wave as soon as the shard is in SBUF
                trigK = nc.gpsimd.trigger_dma(count=None)
                for w in kt_writes:
                    _add_dep_helper(trigK.ins, w.ins, sync=True,
                                    reason="K shard data ready")

                # V wave desc-gen now (Pool is past the K preps)
                prep_wave(v_recv, v_own, range(1, NCORES))

            # V shard -> v_recv slot 0
            v_writes = []
            with tc.tile_pool(name="p1pv" + sfx, bufs=2,
                              space="PSUM") as p1pv:
                for g in range(SLOTS):
                    for e2 in range(2):
                        ps_v = p1pv.tile([128, 384], f32, tag="ps_v")
                        for cc in range(CCH):
                            nc.tensor.matmul(
                                ps_v[:],
                                xt[:, cc, g * 128:(g + 1) * 128],
                                wa_sb[:, cc,
                                      2 * C + 384 * e2:2 * C + 384 * (e2 + 1)],
                                start=(cc == 0), stop=(cc == CCH - 1),
                            )
                        w = nc.vector.tensor_tensor(
                            v_recv[:, 0, g, 6 * e2:6 * (e2 + 1), 0:D],
                            ps_v[:].rearrange("p (h d) -> p h d", d=D),
                            vb_bc[:, 384 * e2:384 * (e2 + 1)].rearrange(
                                "p (h d) -> p h d", d=D),
                            op=ALU.add,
                        )
                        v_writes.append(w)
            trigV = nc.gpsimd.trigger_dma(count=None)
            for w in v_writes:
                _add_dep_helper(trigV.ins, w.ins, sync=True,
                                reason="V shard data ready before transfer")

        # ---- per-slot recv gates ----
        # Each wave bumps the receiver's rsem by 2 (16 // n_dests). Scores
        # need only the K wave (>=2); ctx also needs V' (>=4). Dry-run-only
        # credits (+4) keep the Tile scheduling sim from deadlocking; they
        # are stripped from the compiled program.
        def emit_gate(i, thresh):
            w = nc.tensor.wait_ge(rsems[i], thresh)
            _add_dep_helper(w.ins, credits[i].ins, sync=True,
                            reason="recv gate after sends initiated")
            return w

        credits = {}
        for i in range(1, NCORES):
            fc = nc.gpsimd.nop(nofuse=True).then_inc(rsems[i], 4)
            _add_dep_helper(fc.ins, trigV.ins, sync=False,
                            reason="credit after trigger")
            fake_credits.append((fc, rsems[i]))
            credits[i] = fc

        # ---- phase 2: attention ----
        with (
            tc.tile_pool(name="p2" + sfx, bufs=14) as p2,
            tc.tile_pool(name="p2n" + sfx, bufs=2) as p2n,
            tc.tile_pool(name="p2pc" + sfx, bufs=2, space="PSUM") as p2pc,
        ):
            def ctx_batch(h, i, ps_ctx):
                et = ets[h].pop(i)
                for m in range(SLOTS):
                    n = OWN - 128 * m
                    mm = nc.tensor.matmul(
                        ps_ctx[:, 128 * m:OWN],
                        v_recv[:, i, m, h, :],
                        et[:, OFFS[m]:OFFS[m] + n],
                        start=(i == 0 and m == 0),
                        stop=(i == NCORES - 1 and m == SLOTS - 1),
                        skip_group_check=True,
                    )
                    if i != 0:
                        dep_on(mm, vgate[i])

            def normalize(h, ps_ctx):
                jc = h // 2
                po = 64 * (h % 2)
                rec = p2n.tile([1, OWN], f32, tag="rec")
                nc.vector.reciprocal(rec[:], ps_ctx[D:D + 1, :])
                rb = p2n.tile([D, OWN], f32, tag="rb")
                nc.gpsimd.partition_broadcast(rb[:], rec[:])
                nc.vector.tensor_mul(
                    ctx_sb[po:po + D, jc, :], ps_ctx[0:D, :], rb[:])

            # pass B: per head pair — remaining scores windows, then the
            # ctx batches. Head pairs 4/5 get their own-slot windows here
            # (still from etA: its bufs recycle once early ctx consumes).
            last = H // 2 - 1
            for hp in range(H // 2):
                ha, hb = 2 * hp, 2 * hp + 1
                if hp >= 4:
                    pair_scores(ha, hb, 0, etA)
                ps_ctx_a = p2pc.tile([D + 1, OWN], f32, tag="ps_ctx")
                ps_ctx_b = p2pc.tile([D + 1, OWN], f32, tag="ps_ctx")
                if hp < last:
                    for i in range(1, NCORES):
                        if hp == 0:
                            kgate[i] = emit_gate(i, 2)
                        pair_scores(ha, hb, i, p2)
                    # ctx batches (V ones-column -> row 64 = softmax denom)
                    for i in range(NCORES):
                        if hp == 0 and i >= 1:
                            vgate[i] = emit_gate(i, 4)
                        ctx_batch(ha, i, ps_ctx_a)
                        ctx_batch(hb, i, ps_ctx_b)
                else:
                    # final pair: interleave ctx under the scores/exp
                    # pipeline (V long since arrived) so the serial tail
                    # after the last exp is just one ctx batch + proj
                    for i in range(1, NCORES):
                        pair_scores(ha, hb, i, p2)
                        ctx_batch(ha, i - 1, ps_ctx_a)
                        ctx_batch(hb, i - 1, ps_ctx_b)
                    ctx_batch(ha, NCORES - 1, ps_ctx_a)
                    ctx_batch(hb, NCORES - 1, ps_ctx_b)
                normalize(ha, ps_ctx_a)
                normalize(hb, ps_ctx_b)

        # ---- phase 3: output projection ----
        with (
            tc.tile_pool(name="p3" + sfx, bufs=2) as p3,
            tc.tile_pool(name="p3ps" + sfx, bufs=2, space="PSUM") as p3ps,
        ):
            for ts in range(OWN // 128):
                y_sb = p3.tile([128, C], f32, tag="y_sb")
                for e2 in range(2):
                    ps_o = p3ps.tile([128, 384], f32, tag="ps_o")
                    for cc in range(CCH):
                        nc.tensor.matmul(
                            ps_o[:],
                            ctx_sb[:, cc, ts * 128:(ts + 1) * 128],
                            wp_sb[:, cc, 384 * e2:384 * (e2 + 1)],
                            start=(cc == 0), stop=(cc == CCH - 1),
                        )
                    nc.vector.tensor_tensor(
                        y_sb[:, 384 * e2:384 * (e2 + 1)], ps_o[:],
                        pb_bc[:, 384 * e2:384 * (e2 + 1)], op=ALU.add,
                    )
                nc.gpsimd.dma_start(
                    y_d[ts * 128:(ts + 1) * 128, :], y_sb[:])


_RUNNER = None


def _get_runner():
    """Build the 8-core PJRT executable once; returns f(in_maps) -> results."""
    global _PROGRAM, _RUNNER
    if _RUNNER is not None:
        return _RUNNER
    import jax
    from jax.sharding import Mesh, PartitionSpec
    from jax.experimental.shard_map import shard_map
    from concourse import mybir
    from concourse.bass2jax import (
        _bass_exec_p, install_neuronx_cc_hook, partition_id_tensor)

    if _PROGRAM is None:
        _PROGRAM = _build_program()
    nc = _PROGRAM
    install_neuronx_cc_hook()

    partition_name = (
        nc.partition_id_tensor.name if nc.partition_id_tensor else None)
    in_names, out_names, out_avals, zero_outs = [], [], [], []
    for alloc in nc.m.functions[0].allocations:
        if not isinstance(alloc, mybir.MemoryLocationSet):
            continue
        name = alloc.memorylocations[0].name
        if alloc.kind == "ExternalInput":
            if name == partition_name:
                continue
            in_names.append(name)
        elif alloc.kind == "ExternalOutput":
            shape = tuple(alloc.tensor_shape)
            dtype = mybir.dt.np(alloc.dtype)
            out_names.append(name)
            out_avals.append(jax.core.ShapedArray(shape, dtype))
            zero_outs.append(np.zeros(shape, dtype))
    n_params = len(in_names)
    all_names = in_names + out_names
    if partition_name is not None:
        all_names = all_names + [partition_name]
    donate = tuple(range(n_params, n_params + len(out_names)))

    def _body(*args):
        operands = list(args)
        if partition_name is not None:
            operands.append(partition_id_tensor())
        outs = _bass_exec_p.bind(
            *operands,
            out_avals=tuple(out_avals),
            in_names=tuple(all_names),
            out_names=tuple(out_names),
            lowering_input_output_aliases=(),
            sim_require_finite=True,
            sim_require_nnan=True,
            nc=nc,
        )
        return tuple(outs)

    devices = jax.devices()[:NCORES]
    mesh = Mesh(np.asarray(devices), ("core",))
    specs = (PartitionSpec("core"),) * (n_params + len(out_names))
    sharded = jax.jit(
        shard_map(_body, mesh=mesh, in_specs=specs,
                  out_specs=(PartitionSpec("core"),) * len(out_names),
                  check_rep=False),
        donate_argnums=donate, keep_unused=True,
    )

    def run(in_maps):
        concat_in = [
            np.concatenate([np.asarray(m[name]) for m in in_maps], axis=0)
            for name in in_names
        ]
        concat_zeros = [
            np.zeros((NCORES * z.shape[0], *z.shape[1:]), z.dtype)
            for z in zero_outs
        ]
        out_arrs = jax.block_until_ready(sharded(*concat_in, *concat_zeros))
        return [
            {name: np.asarray(out_arrs[i]).reshape(NCORES, *out_avals[i].shape)[c]
             for i, name in enumerate(out_names)}
            for c in range(NCORES)
        ]

    _RUNNER = run
    run._parts = (_body, in_names, out_names, out_avals, zero_outs, mesh)
    return run


def _make_timed_fn(nc, in_maps):
    """Compile one-dispatch callable with device-resident inputs."""
    import jax
    from jax.sharding import Mesh, NamedSharding, PartitionSpec
    from jax.experimental.shard_map import shard_map
    from concourse import mybir
    from concourse.bass2jax import (
        _bass_exec_p, install_neuronx_cc_hook, partition_id_tensor)

    install_neuronx_cc_hook()
    partition_name = (
        nc.partition_id_tensor.name if nc.partition_id_tensor else None)
    in_names, out_names, out_avals, zero_outs = [], [], [], []
    for alloc in nc.m.functions[0].allocations:
        if not isinstance(alloc, mybir.MemoryLocationSet):
            continue
        name = alloc.memorylocations[0].name
        if alloc.kind == "ExternalInput":
            if name == partition_name:
                continue
            in_names.append(name)
        elif alloc.kind == "ExternalOutput":
            out_names.append(name)
            out_avals.append(jax.core.ShapedArray(
                tuple(alloc.tensor_shape), mybir.dt.np(alloc.dtype)))
            zero_outs.append(np.zeros(
                tuple(alloc.tensor_shape), mybir.dt.np(alloc.dtype)))
    all_names = in_names + out_names
    if partition_name is not None:
        all_names = all_names + [partition_name]

    def _body(*args):
        operands = list(args)
        if partition_name is not None:
            operands.append(partition_id_tensor())
        return tuple(_bass_exec_p.bind(
            *operands,
            out_avals=tuple(out_avals),
            in_names=tuple(all_names),
            out_names=tuple(out_names),
            lowering_input_output_aliases=(),
            sim_require_finite=True, sim_require_nnan=True, nc=nc,
        ))

    devices = jax.devices()[:NCORES]
    mesh = Mesh(np.asarray(devices), ("core",))
    fn = jax.jit(shard_map(
        _body, mesh=mesh,
        in_specs=(PartitionSpec("core"),) * (len(in_names) + len(zero_outs)),
        out_specs=(PartitionSpec("core"),) * len(out_names),
        check_rep=False))
    sh = NamedSharding(mesh, PartitionSpec("core"))
    concat_in = [
        jax.device_put(np.concatenate(
            [np.asarray(m[name]) for m in in_maps], axis=0), sh)
        for name in in_names
    ]
    concat_zeros = [
        jax.device_put(
            np.zeros((NCORES * z.shape[0], *z.shape[1:]), z.dtype), sh)
        for z in zero_outs
    ]
    jax.block_until_ready(fn(*concat_in, *concat_zeros))  # warm/compile

    def call():
        import time
        t0 = time.perf_counter()
        jax.block_until_ready(fn(*concat_in, *concat_zeros))
        return time.perf_counter() - t0
    return call


def _bench_device_time(in_maps, iters=20, n_rep=8):
    """Per-execution device time: the kernel body emitted n_rep times in one
    program vs once. The n_rep bodies are serialized on-device (DRAM chain),
    so min(walls_n) - min(walls_1) is a tight estimate of (n_rep-1) execs
    that is robust to dispatch-RTT congestion spikes."""
    global _PROGRAM
    if _PROGRAM is None:
        _PROGRAM = _build_program()
    call1 = _make_timed_fn(_PROGRAM, in_maps)
    calln = _make_timed_fn(_build_program(n_rep), in_maps)

    w1, wn = [], []
    for _ in range(max(4, iters // 3)):
        call1()  # absorb NEFF swap from previous block
        w1.extend(call1() for _ in range(6))
        calln()
        wn.extend(calln() for _ in range(6))
    per_exec = (min(wn) - min(w1)) / (n_rep - 1)
    deltas = sorted(wn)[:3]
    return per_exec, {
        "rep1_min": min(w1),
        f"rep{n_rep}_min": min(wn),
        "delta_med": per_exec,
        "rep1_p25": sorted(w1)[len(w1) // 4],
        f"rep{n_rep}_p25": sorted(wn)[len(wn) // 4],
    }


def _make_in_maps(x2, wa, ba, wp, bp):
    import ml_dtypes
    bf = ml_dtypes.bfloat16
    wa16 = np.ascontiguousarray(wa.astype(bf))
    wp16 = np.ascontiguousarray(wp.astype(bf))
    in_maps = []
    for c in range(NCORES):
        xo = np.concatenate([x2[128 * b:128 * (b + 1)] for b in _blocks(c)], 0)
        in_maps.append({
            "x_own": np.ascontiguousarray(xo),
            "W_attn": wa16, "b_attn": ba, "W_proj": wp16, "b_proj": bp,
            "masks": _build_masks(c).astype(bf),
        })
    return in_maps


def kernel(x, W_attn, b_attn, W_proj, b_proj):
    x2 = np.ascontiguousarray(np.asarray(x, dtype=np.float32).reshape(T, C))
    wa = np.ascontiguousarray(np.asarray(W_attn, dtype=np.float32))
    ba = np.ascontiguousarray(np.asarray(b_attn, dtype=np.float32))
    wp = np.ascontiguousarray(np.asarray(W_proj, dtype=np.float32))
    bp = np.ascontiguousarray(np.asarray(b_proj, dtype=np.float32))

    run = _get_runner()
    res = run(_make_in_maps(x2, wa, ba, wp, bp))

    y = np.empty((T, C), dtype=np.float32)
    for c in range(NCORES):
        yo = res[c]["y_own"]
        for s, b in enumerate(_blocks(c)):
            y[128 * b:128 * (b + 1)] = yo[128 * s:128 * (s + 1)]
    return y.reshape(1, T, C)


# revision 23
# speedup vs baseline: 1.8806x; 1.4374x over previous
"""Causal attention (B=1, T=4096, C=768, H=12, D=64) on 8 trn2 NeuronCores.

v2: ACT-bound redesign. Sharding as v1 (32 blocks of 128 rows; core r owns
blocks {r, r+8, r+16, r+24} as query and K/V rows; XOR-permuted K^T / V'
shard exchange via remote_dma_broadcast, masks absorb the permutation).

Differences vs v1:
- One wide exp per (head, slot): the 4 causal-class score matmuls land in a
  contiguous 3-bank PSUM region ([0:512],[512:896],[896:1024],[1024:1280])
  so softmax is a single [128,1280] Activation instead of 3 narrow ones.
- Every non-exp op leaves the scalar engine: x^T copies and Q/K bias adds
  run on DVE (tensor_scalar_add), V/proj biases stay on DVE.
- Remote-DMA descriptor generation is issued at kernel start (Pool is idle
  then); trigger_dma fires the K wave as soon as K^T is computed and the V
  wave after V', so transfers overlap phase-1 compute.
- Split gates: scores matmuls wait rsem>=2 (K wave), ctx matmuls rsem>=4
  (V wave). Per head: 8 scores/exp windows then 8 ctx batches, with a deep
  et pool so ACT can run ahead of the V exchange.
"""

import numpy as np

T = 4096
C = 768
H = 12
D = 64
J3 = 3 * C          # 2304
NCORES = 8
QB = 128            # query block rows
NQB = T // QB       # 32
SLOTS = 4           # owned query blocks per core (classes m=0..3)
OWN = SLOTS * QB    # 512
CCH = C // 128      # 6 contraction chunks

# class m -> offset of its score strip inside the [128, 1280] region
OFFS = {0: 0, 1: 512, 3: 896, 2: 1024}

_PROGRAM = None


def _blocks(c):
    return [c, 8 + c, 16 + c, 24 + c]


def _build_masks(r):
    """[128, 8*128] f32: mask[k, i*128+q] for diagonal-class key slot i.

    Receiver r's slot i holds sender j = r^i. Diagonal class m==s: key block
    8m+j vs query block 8m+r -> keep iff (r-j)*128 + q - k >= 0.
    """
    k = np.arange(128)[:, None]
    q = np.arange(128)[None, :]
    cols = []
    for i in range(8):
        j = r ^ i
        blk = (((r - j) * 128 + q - k) >= 0).astype(np.float32)
        cols.append(np.concatenate([blk, blk], axis=1))  # doubled: m3+m2 merge
    return np.concatenate(cols, axis=1)


def _build_program(repeat=1):
    import concourse.bass as bass
    import concourse.tile as tile
    from concourse import bacc, mybir
    from concourse.bass import _add_dep_helper
    from concourse.masks import make_identity
    from contextlib import ExitStack

    f32 = mybir.dt.float32
    bf16 = mybir.dt.bfloat16

    nc = bacc.Bacc(
        "TRN2", target_bir_lowering=False, debug=False,
        enable_asserts=False, num_devices=NCORES,
    )

    xo_d = nc.dram_tensor("x_own", [OWN, C], f32, kind="ExternalInput")
    wa_d = nc.dram_tensor("W_attn", [C, J3], bf16, kind="ExternalInput")
    ba_d = nc.dram_tensor("b_attn", [J3], f32, kind="ExternalInput")
    wp_d = nc.dram_tensor("W_proj", [C, C], bf16, kind="ExternalInput")
    bp_d = nc.dram_tensor("b_proj", [C], f32, kind="ExternalInput")
    mk_d = nc.dram_tensor("masks", [128, 8 * 256], bf16, kind="ExternalInput")
    y_d = nc.dram_tensor("y_own", [OWN, C], f32, kind="ExternalOutput")

    fake_credits = []

    with tile.TileContext(nc) as tc:
        for it in range(repeat):
            _emit_once(nc, tc, it, xo_d, wa_d, ba_d, wp_d, bp_d, mk_d, y_d,
                       fake_credits)

    nc.compile()

    # Strip the dry-run-only rsem credits: on hardware the recv gates must be
    # satisfied by the peers' remote increments alone.
    for fake_credit, rsem in fake_credits:
        si = fake_credit.ins.sync_info
        assert si is not None and si.on_update
        kept = [u for u in si.on_update if u.id != rsem.num]
        assert len(kept) < len(si.on_update), "rsem credit not found to strip"
        si.on_update = kept
    return nc


def _emit_once(nc, tc, it, xo_d, wa_d, ba_d, wp_d, bp_d, mk_d, y_d,
               fake_credits):
    import concourse.tile as tile
    from concourse import mybir
    from concourse.bass import _add_dep_helper
    from concourse.masks import make_identity
    from contextlib import ExitStack

    f32 = mybir.dt.float32
    bf16 = mybir.dt.bfloat16
    AF = mybir.ActivationFunctionType
    ALU = mybir.AluOpType
    sfx = f"_{it}"

    rsems = {i: nc.alloc_semaphore(f"rsem{i}" + sfx)
             for i in range(1, NCORES)}
    lsem = nc.alloc_semaphore("lsem" + sfx)  # local send-completion

    with ExitStack() as ctx:
        per = ctx.enter_context(tc.tile_pool(name="per" + sfx, bufs=1))

        # ---- persistent tiles ----
        wp_sb = per.tile([128, CCH, C], bf16)
        # K^T shards: [d(2 heads), slot i, jc(head pair), class m, 128]
        kt_recv = per.tile([128, NCORES, CCH, SLOTS, 128], bf16)
        # V' shards: [k, slot i, class m, head, 65]; [...,64] = 1.0
        v_recv = per.tile([128, NCORES, SLOTS, H, D + 1], bf16)
        qt_sb = per.tile([128, CCH, OWN], bf16)
        ctx_sb = per.tile([128, CCH, OWN], bf16)
        mask_sb = per.tile([128, 8 * 256], bf16)
        ba_sb = per.tile([128, J3 // 128], f32)
        vb_bc = per.tile([128, C], f32)
        pb_bc = per.tile([128, C], f32)
        ident = per.tile([128, 128], f32)
        vb_st = per.tile([1, C], f32)
        pb_st = per.tile([1, C], f32)

        make_identity(nc, ident[:])
        # ones column of own V' shard (slot 0); peers receive it via bcast
        nc.gpsimd.memset(v_recv[:, 0, :, :, D:D + 1], 1.0)

        # ---- small DMAs + x ----
        nc.sync.dma_start(ba_sb[:], ba_d.rearrange("(a p) -> p a", p=128))
        nc.sync.dma_start(vb_st[:], ba_d[2 * C:3 * C][None, :])
        if it > 0:
            # benchmark-only serialization: body `it` consumes (x0) the
            # previous body's output, so repeats can't overlap or fold.
            # gpsimd queue keeps the SP queue pure-input.
            chain = per.tile([1, C], f32)
            nc.gpsimd.dma_start(chain[:], y_d[0:1, :])
            nc.vector.tensor_scalar_mul(chain[:], chain[:], 0.0)
            nc.vector.tensor_add(vb_st[:], vb_st[:], chain[:])
        nc.sync.dma_start(pb_st[:], bp_d[None, :])
        nc.sync.dma_start(mask_sb[:], mk_d[:])

        # ---- early desc-gen: K wave (slot i gets sender r^i) ----
        kt_own = kt_recv[:, 0, :, :, :].rearrange("p a b c -> p (a b c)")
        v_own = v_recv[:, 0, :, :, :].rearrange("p a b c -> p (a b c)")

        def prep_wave(recv, own, slots):
            for i in slots:
                rd = [None] * 8
                rd[i] = (0, i)
                nc.gpsimd.remote_dma_broadcast(
                    recv[:, i, :, :, :].rearrange("p a b c -> p (a b c)"),
                    own, rsems[i], lsem, rdests=rd)

        prep_wave(kt_recv, kt_own, range(1, NCORES))

        nc.gpsimd.partition_broadcast(vb_bc[:], vb_st[:])
        nc.gpsimd.partition_broadcast(pb_bc[:], pb_st[:])

        # score-region PSUM pool (2 x 3 banks) + pass-A et pool live for the
        # whole kernel so own-slot attention can interleave into phase 1
        p2ps = ctx.enter_context(
            tc.tile_pool(name="p2ps" + sfx, bufs=2, space="PSUM"))
        etA = ctx.enter_context(tc.tile_pool(name="etA" + sfx, bufs=8))

        kgate = {}
        vgate = {}
        scale = 1.0 / float(np.sqrt(D))

        def dep_on(inst, gate):
            # same-engine (PE) ordering edge: the gate precedes every
            # attention matmul reading the remote shard
            _add_dep_helper(inst.ins, gate.ins, sync=False,
                            reason="attention reads remote K/V shard")

        def pair_scores(ha, hb, i, et_pool):
            """Score windows for both heads of a pair at slot i.

            The heads live on disjoint partition halves (po 0 / 64), so
            their K=64-contraction matmuls target disjoint PE row groups
            (tile_position auto-derived from base_partition) and execute
            concurrently when interleaved in the issue stream.
            """
            jc = ha // 2
            reg_a = p2ps.tile([128, 1536], f32, tag="reg")
            reg_b = p2ps.tile([128, 1536], f32, tag="reg")
            regs = (reg_a, reg_b)
            for m in range(SLOTS):
                n = OWN - 128 * m
                for h, reg in zip((ha, hb), regs):
                    po = 64 * (h % 2)
                    if i == 0 and jc < 4:
                        # own-slot K^T comes from the staging tile: it is
                        # written with no WAR against the K-wave desc-gens,
                        # so these windows start ~15us earlier
                        kt_src = stash["kt_st"][po:po + D, jc, m, :]
                    else:
                        kt_src = kt_recv[po:po + D, i, jc, m, :]
                    mm = nc.tensor.matmul(
                        reg[:, OFFS[m]:OFFS[m] + n],
                        kt_src,
                        qt_sb[po:po + D, jc, 128 * m:OWN],
                        start=True, stop=True,
                    )
                    if i != 0:
                        dep_on(mm, kgate[i])
            out = []
            for h, reg in zip((ha, hb), regs):
                et = et_pool.tile([128, 1280], bf16, tag="et")
                nc.scalar.activation(
                    et[:], reg[:, 0:1280], AF.Exp, scale=scale)
                for lo, wdt in ((OFFS[0], 128), (OFFS[1], 128),
                                (OFFS[3], 256)):
                    # diagonal strips (query slot == class); m3+m2 strips
                    # are adjacent at [896:1152] -> one op on a doubled mask
                    nc.vector.tensor_mul(
                        et[:, lo:lo + wdt], et[:, lo:lo + wdt],
                        mask_sb[:, i * 256:i * 256 + wdt])
                ets[h][i] = et
                out.append(et)
            return out

        ets = {h: {} for h in range(H)}
        stash = {}

        # ---- phase 1: own-row QKV projection ----
        with tc.tile_pool(name="p1" + sfx, bufs=1) as p1:
            # x first on the DMA queue (transposes gate everything), then
            # W_attn in K / Q / V column groups, W_proj last (phase 3 only)
            xst = p1.tile([128, SLOTS, C], f32, tag="xst")
            xre = xo_d.rearrange("(g p) c -> p g c", p=128)
            for cc in range(CCH):
                nc.sync.dma_start(
                    xst[:, :, cc * 128:(cc + 1) * 128],
                    xre[:, :, cc * 128:(cc + 1) * 128])
            wa_sb = p1.tile([128, CCH, J3], bf16, tag="wa_sb")
            for cg in (1, 0, 2):  # column group: k, q, v
                for cc in range(CCH):
                    nc.sync.dma_start(
                        wa_sb[:, cc, cg * C:(cg + 1) * C],
                        wa_d[cc * 128:(cc + 1) * 128, cg * C:(cg + 1) * C])
            for cc in range(CCH):
                nc.sync.dma_start(
                    wp_sb[:, cc, :], wp_d[cc * 128:(cc + 1) * 128, :])
            xt = p1.tile([128, CCH, OWN], bf16, tag="xt")

            # staging for K^T chunks jc<4: written WAR-free (the K-wave
            # desc-gens only read kt_recv), feeds pass-A windows immediately
            kt_st = p1.tile([128, 4, SLOTS, 128], bf16, tag="kt_st")
            stash["kt_st"] = kt_st

            with tc.tile_pool(name="p1ps" + sfx, bufs=2,
                              space="PSUM") as p1ps:
                for cc in range(CCH):
                    ps_t = p1ps.tile([128, OWN], f32, tag="ps")
                    for g in range(SLOTS):
                        nc.tensor.transpose(
                            ps_t[:, g * 128:(g + 1) * 128],
                            xst[:, g, cc * 128:(cc + 1) * 128],
                            ident[:],
                        )
                    nc.vector.tensor_scalar_add(xt[:, cc, :], ps_t[:], 0.0)

                # K^T, Q^T and pass-A own-slot windows, chunk-interleaved:
                # each jc enables head pair jc's window right away, so the
                # exp pipeline starts ~chunk-0 time instead of after all of
                # phase 1
                kt_writes = []
                for jc in range(CCH):
                    ps_k = p1ps.tile([128, OWN], f32, tag="ps")
                    for cc in range(CCH):
                        nc.tensor.matmul(
                            ps_k[:],
                            wa_sb[:, cc,
                                  (CCH + jc) * 128:(CCH + jc + 1) * 128],
                            xt[:, cc, :],
                            start=(cc == 0), stop=(cc == CCH - 1),
                        )
                    if jc < 4:
                        nc.vector.tensor_scalar_add(
                            kt_st[:, jc, :, :].rearrange("p a b -> p (a b)"),
                            ps_k[:], ba_sb[:, CCH + jc:CCH + jc + 1])
                    else:
                        w = nc.vector.tensor_scalar_add(
                            kt_recv[:, 0, jc, :, :].rearrange(
                                "p a b -> p (a b)"),
                            ps_k[:], ba_sb[:, CCH + jc:CCH + jc + 1])
                        kt_writes.append(w)
                    ps_q = p1ps.tile([128, OWN], f32, tag="ps")
                    for cc in range(CCH):
                        nc.tensor.matmul(
                            ps_q[:],
                            wa_sb[:, cc, jc * 128:(jc + 1) * 128],
                            xt[:, cc, :],
                            start=(cc == 0), stop=(cc == CCH - 1),
                        )
                    nc.vector.tensor_scalar_add(
                        qt_sb[:, jc, :], ps_q[:], ba_sb[:, jc:jc + 1])
                    if jc < 4:
                        pair_scores(2 * jc, 2 * jc + 1, 0, etA)

                # exchange copies of the staged chunks (these pay the WAR
                # against the K-wave desc-gens, off the critical path)
                for jc in range(4):
                    w = nc.vector.tensor_scalar_add(
                        kt_recv[:, 0, jc, :, :].rearrange("p a b -> p (a b)"),
                        kt_st[:, jc, :, :].rearrange("p a b -> p (a b)"),
                        0.0)
                    kt_writes.append(w)
                # fire the K wave as soon as the shard is in SBUF
                trigK = nc.gpsimd.trigger_dma(count=None)
                for w in kt_writes:
                    _add_dep_helper(trigK.ins, w.ins, sync=True,
                                    reason="K shard data ready")

                # V wave desc-gen now (Pool is past the K preps)
                prep_wave(v_recv, v_own, range(1, NCORES))

            # V shard -> v_recv slot 0
            v_writes = []
            with tc.tile_pool(name="p1pv" + sfx, bufs=2,
                              space="PSUM") as p1pv:
                for g in range(SLOTS):
                    for e2 in range(2):
                        ps_v = p1pv.tile([128, 384], f32, tag="ps_v")
                        for cc in range(CCH):
                            nc.tensor.matmul(
                                ps_v[:],
                                xt[:, cc, g * 128:(g + 1) * 128],
                                wa_sb[:, cc,
                                      2 * C + 384 * e2:2 * C + 384 * (e2 + 1)],
                                start=(cc == 0), stop=(cc == CCH - 1),
                            )
                        w = nc.vector.tensor_tensor(
                            v_recv[:, 0, g, 6 * e2:6 * (e2 + 1), 0:D],
                            ps_v[:].rearrange("p (h d) -> p h d", d=D),
                            vb_bc[:, 384 * e2:384 * (e2 + 1)].rearrange(
                                "p (h d) -> p h d", d=D),
                            op=ALU.add,
                        )
                        v_writes.append(w)
            trigV = nc.gpsimd.trigger_dma(count=None)
            for w in v_writes:
                _add_dep_helper(trigV.ins, w.ins, sync=True,
                                reason="V shard data ready before transfer")

        # ---- per-slot recv gates ----
        # Each wave bumps the receiver's rsem by 2 (16 // n_dests). Scores
        # need only the K wave (>=2); ctx also needs V' (>=4). Dry-run-only
        # credits (+4) keep the Tile scheduling sim from deadlocking; they
        # are stripped from the compiled program.
        def emit_gate(i, thresh):
            w = nc.tensor.wait_ge(rsems[i], thresh)
            _add_dep_helper(w.ins, credits[i].ins, sync=True,
                            reason="recv gate after sends initiated")
            return w

        credits = {}
        for i in range(1, NCORES):
            fc = nc.gpsimd.nop(nofuse=True).then_inc(rsems[i], 4)
            _add_dep_helper(fc.ins, trigV.ins, sync=False,
                            reason="credit after trigger")
            fake_credits.append((fc, rsems[i]))
            credits[i] = fc

        # ---- phase 2: attention ----
        with (
            tc.tile_pool(name="p2" + sfx, bufs=14) as p2,
            tc.tile_pool(name="p2n" + sfx, bufs=2) as p2n,
            tc.tile_pool(name="p2pc" + sfx, bufs=2, space="PSUM") as p2pc,
        ):
            def ctx_batch(h, i, ps_ctx):
                et = ets[h].pop(i)
                for m in range(SLOTS):
                    n = OWN - 128 * m
                    mm = nc.tensor.matmul(
                        ps_ctx[:, 128 * m:OWN],
                        v_recv[:, i, m, h, :],
                        et[:, OFFS[m]:OFFS[m] + n],
                        start=(i == 0 and m == 0),
                        stop=(i == NCORES - 1 and m == SLOTS - 1),
                        skip_group_check=True,
                    )
                    if i != 0:
                        dep_on(mm, vgate[i])

            def normalize(h, ps_ctx):
                jc = h // 2
                po = 64 * (h % 2)
                rec = p2n.tile([1, OWN], f32, tag="rec")
                nc.vector.reciprocal(rec[:], ps_ctx[D:D + 1, :])
                rb = p2n.tile([D, OWN], f32, tag="rb")
                nc.gpsimd.partition_broadcast(rb[:], rec[:])
                nc.vector.tensor_mul(
                    ctx_sb[po:po + D, jc, :], ps_ctx[0:D, :], rb[:])

            # pass B: per head pair — remaining scores windows, then the
            # ctx batches. Head pairs 4/5 get their own-slot windows here
            # (still from etA: its bufs recycle once early ctx consumes).
            last = H // 2 - 1
            for hp in range(H // 2):
                ha, hb = 2 * hp, 2 * hp + 1
                if hp >= 4:
                    pair_scores(ha, hb, 0, etA)
                ps_ctx_a = p2pc.tile([D + 1, OWN], f32, tag="ps_ctx")
                ps_ctx_b = p2pc.tile([D + 1, OWN], f32, tag="ps_ctx")
                if hp < last:
                    for i in range(1, NCORES):
                        if hp == 0:
                            kgate[i] = emit_gate(i, 2)
                        pair_scores(ha, hb, i, p2)
                    # ctx batches (V ones-column -> row 64 = softmax denom)
                    for i in range(NCORES):
                        if hp == 0 and i >= 1:
                            vgate[i] = emit_gate(i, 4)
                        ctx_batch(ha, i, ps_ctx_a)
                        ctx_batch(hb, i, ps_ctx_b)
                else:
                    # final pair: interleave ctx under the scores/exp
                    # pipeline (V long since arrived) so the serial tail
                    # after the last exp is just one ctx batch + proj
                    for i in range(1, NCORES):
                        pair_scores(ha, hb, i, p2)
                        ctx_batch(ha, i - 1, ps_ctx_a)
                        ctx_batch(hb, i - 1, ps_ctx_b)
                    ctx_batch(ha, NCORES - 1, ps_ctx_a)
                    ctx_batch(hb, NCORES - 1, ps_ctx_b)
                normalize(ha, ps_ctx_a)
                normalize(hb, ps_ctx_b)

        # ---- phase 3: output projection ----
        with (
            tc.tile_pool(name="p3" + sfx, bufs=2) as p3,
            tc.tile_pool(name="p3ps" + sfx, bufs=2, space="PSUM") as p3ps,
        ):
            for ts in range(OWN // 128):
                y_sb = p3.tile([128, C], f32, tag="y_sb")
                for e2 in range(2):
                    ps_o = p3ps.tile([128, 384], f32, tag="ps_o")
                    for cc in range(CCH):
                        nc.tensor.matmul(
                            ps_o[:],
                            ctx_sb[:, cc, ts * 128:(ts + 1) * 128],
                            wp_sb[:, cc, 384 * e2:384 * (e2 + 1)],
                            start=(cc == 0), stop=(cc == CCH - 1),
                        )
                    nc.vector.tensor_tensor(
                        y_sb[:, 384 * e2:384 * (e2 + 1)], ps_o[:],
                        pb_bc[:, 384 * e2:384 * (e2 + 1)], op=ALU.add,
                    )
                nc.gpsimd.dma_start(
                    y_d[ts * 128:(ts + 1) * 128, :], y_sb[:])


_RUNNER = None


def _get_runner():
    """Build the 8-core PJRT executable once; returns f(in_maps) -> results."""
    global _PROGRAM, _RUNNER
    if _RUNNER is not None:
        return _RUNNER
    import jax
    from jax.sharding import Mesh, PartitionSpec
    from jax.experimental.shard_map import shard_map
    from concourse import mybir
    from concourse.bass2jax import (
        _bass_exec_p, install_neuronx_cc_hook, partition_id_tensor)

    if _PROGRAM is None:
        _PROGRAM = _build_program()
    nc = _PROGRAM
    install_neuronx_cc_hook()

    partition_name = (
        nc.partition_id_tensor.name if nc.partition_id_tensor else None)
    in_names, out_names, out_avals, zero_outs = [], [], [], []
    for alloc in nc.m.functions[0].allocations:
        if not isinstance(alloc, mybir.MemoryLocationSet):
            continue
        name = alloc.memorylocations[0].name
        if alloc.kind == "ExternalInput":
            if name == partition_name:
                continue
            in_names.append(name)
        elif alloc.kind == "ExternalOutput":
            shape = tuple(alloc.tensor_shape)
            dtype = mybir.dt.np(alloc.dtype)
            out_names.append(name)
            out_avals.append(jax.core.ShapedArray(shape, dtype))
            zero_outs.append(np.zeros(shape, dtype))
    n_params = len(in_names)
    all_names = in_names + out_names
    if partition_name is not None:
        all_names = all_names + [partition_name]
    donate = tuple(range(n_params, n_params + len(out_names)))

    def _body(*args):
        operands = list(args)
        if partition_name is not None:
            operands.append(partition_id_tensor())
        outs = _bass_exec_p.bind(
            *operands,
            out_avals=tuple(out_avals),
            in_names=tuple(all_names),
            out_names=tuple(out_names),
            lowering_input_output_aliases=(),
            sim_require_finite=True,
            sim_require_nnan=True,
            nc=nc,
        )
        return tuple(outs)

    devices = jax.devices()[:NCORES]
    mesh = Mesh(np.asarray(devices), ("core",))
    specs = (PartitionSpec("core"),) * (n_params + len(out_names))
    sharded = jax.jit(
        shard_map(_body, mesh=mesh, in_specs=specs,
                  out_specs=(PartitionSpec("core"),) * len(out_names),
                  check_rep=False),
        donate_argnums=donate, keep_unused=True,
    )

    def run(in_maps):
        concat_in = [
            np.concatenate([np.asarray(m[name]) for m in in_maps], axis=0)
            for name in in_names
        ]
        concat_zeros = [
            np.zeros((NCORES * z.shape[0], *z.shape[1:]), z.dtype)
            for z in zero_outs
        ]
        out_arrs = jax.block_until_ready(sharded(*concat_in, *concat_zeros))
        return [
            {name: np.asarray(out_arrs[i]).reshape(NCORES, *out_avals[i].shape)[c]
             for i, name in enumerate(out_names)}
            for c in range(NCORES)
        ]

    _RUNNER = run
    run._parts = (_body, in_names, out_names, out_avals, zero_outs, mesh)
    return run


def _make_timed_fn(nc, in_maps):
    """Compile one-dispatch callable with device-resident inputs."""
    import jax
    from jax.sharding import Mesh, NamedSharding, PartitionSpec
    from jax.experimental.shard_map import shard_map
    from concourse import mybir
    from concourse.bass2jax import (
        _bass_exec_p, install_neuronx_cc_hook, partition_id_tensor)

    install_neuronx_cc_hook()
    partition_name = (
        nc.partition_id_tensor.name if nc.partition_id_tensor else None)
    in_names, out_names, out_avals, zero_outs = [], [], [], []
    for alloc in nc.m.functions[0].allocations:
        if not isinstance(alloc, mybir.MemoryLocationSet):
            continue
        name = alloc.memorylocations[0].name
        if alloc.kind == "ExternalInput":
            if name == partition_name:
                continue
            in_names.append(name)
        elif alloc.kind == "ExternalOutput":
            out_names.append(name)
            out_avals.append(jax.core.ShapedArray(
                tuple(alloc.tensor_shape), mybir.dt.np(alloc.dtype)))
            zero_outs.append(np.zeros(
                tuple(alloc.tensor_shape), mybir.dt.np(alloc.dtype)))
    all_names = in_names + out_names
    if partition_name is not None:
        all_names = all_names + [partition_name]

    def _body(*args):
        operands = list(args)
        if partition_name is not None:
            operands.append(partition_id_tensor())
        return tuple(_bass_exec_p.bind(
            *operands,
            out_avals=tuple(out_avals),
            in_names=tuple(all_names),
            out_names=tuple(out_names),
            lowering_input_output_aliases=(),
            sim_require_finite=True, sim_require_nnan=True, nc=nc,
        ))

    devices = jax.devices()[:NCORES]
    mesh = Mesh(np.asarray(devices), ("core",))
    fn = jax.jit(shard_map(
        _body, mesh=mesh,
        in_specs=(PartitionSpec("core"),) * (len(in_names) + len(zero_outs)),
        out_specs=(PartitionSpec("core"),) * len(out_names),
        check_rep=False))
    sh = NamedSharding(mesh, PartitionSpec("core"))
    concat_in = [
        jax.device_put(np.concatenate(
            [np.asarray(m[name]) for m in in_maps], axis=0), sh)
        for name in in_names
    ]
    concat_zeros = [
        jax.device_put(
            np.zeros((NCORES * z.shape[0], *z.shape[1:]), z.dtype), sh)
        for z in zero_outs
    ]
    jax.block_until_ready(fn(*concat_in, *concat_zeros))  # warm/compile

    def call():
        import time
        t0 = time.perf_counter()
        jax.block_until_ready(fn(*concat_in, *concat_zeros))
        return time.perf_counter() - t0
    return call


def _bench_device_time(in_maps, iters=30, n_rep=8):
    """Per-execution device time: the kernel body emitted n_rep times in one
    program vs once. The n_rep bodies are serialized on-device (DRAM chain),
    so min(walls_n) - min(walls_1) is a tight estimate of (n_rep-1) execs
    that is robust to dispatch-RTT congestion spikes. A lone freak-low wall
    (tunnel anomaly) is discarded: if the min sits far below the next
    samples, the second-smallest is used instead."""
    global _PROGRAM
    if _PROGRAM is None:
        _PROGRAM = _build_program()
    call1 = _make_timed_fn(_PROGRAM, in_maps)
    calln = _make_timed_fn(_build_program(n_rep), in_maps)

    w1, wn = [], []
    for _ in range(max(4, iters // 3)):
        call1()  # absorb NEFF swap from previous block
        w1.extend(call1() for _ in range(6))
        calln()
        wn.extend(calln() for _ in range(6))

    def floor_of(ws):
        s = sorted(ws)
        # an isolated minimum >2ms below the following two samples is a
        # dispatch-path anomaly, not the true floor
        if s[0] < s[1] - 2e-3 and s[0] < s[2] - 2e-3:
            return s[1]
        return s[0]

    per_exec = (floor_of(wn) - floor_of(w1)) / (n_rep - 1)
    return per_exec, {
        "rep1_min": min(w1),
        f"rep{n_rep}_min": min(wn),
        "rep1_floor": floor_of(w1),
        f"rep{n_rep}_floor": floor_of(wn),
        "delta_med": per_exec,
        "rep1_p25": sorted(w1)[len(w1) // 4],
        f"rep{n_rep}_p25": sorted(wn)[len(wn) // 4],
    }


def _make_in_maps(x2, wa, ba, wp, bp):
    import ml_dtypes
    bf = ml_dtypes.bfloat16
    wa16 = np.ascontiguousarray(wa.astype(bf))
    wp16 = np.ascontiguousarray(wp.astype(bf))
    in_maps = []
    for c in range(NCORES):
        xo = np.concatenate([x2[128 * b:128 * (b + 1)] for b in _blocks(c)], 0)
        in_maps.append({
            "x_own": np.ascontiguousarray(xo),
            "W_attn": wa16, "b_attn": ba, "W_proj": wp16, "b_proj": bp,
            "masks": _build_masks(c).astype(bf),
        })
    return in_maps


def kernel(x, W_attn, b_attn, W_proj, b_proj):
    x2 = np.ascontiguousarray(np.asarray(x, dtype=np.float32).reshape(T, C))
    wa = np.ascontiguousarray(np.asarray(W_attn, dtype=np.float32))
    ba = np.ascontiguousarray(np.asarray(b_attn, dtype=np.float32))
    wp = np.ascontiguousarray(np.asarray(W_proj, dtype=np.float32))
    bp = np.ascontiguousarray(np.asarray(b_proj, dtype=np.float32))

    run = _get_runner()
    res = run(_make_in_maps(x2, wa, ba, wp, bp))

    y = np.empty((T, C), dtype=np.float32)
    for c in range(NCORES):
        yo = res[c]["y_own"]
        for s, b in enumerate(_blocks(c)):
            y[128 * b:128 * (b + 1)] = yo[128 * s:128 * (s + 1)]
    return y.reshape(1, T, C)
